# revision 6
# baseline (speedup 1.0000x reference)
"""DeepHamCritic (3x GCNConv + dense head) on 8 trn2 NeuronCores.

Strategy (v2):
  - Host: dense normalized adjacency A [1000,1000] from edge_index
    (self-loops + deg^-1/2 symmetric norm), pad 1000 -> 1024, fp16.
  - GCN fp16, sharded by destination node (128 per core): aggregation as
    PE matmuls against the local A^T column slice; fp16 AllGather of node
    features between layers; fp32 PSUM accumulate throughout.
  - Dense head: Wd1 [512000,256] row-sharded, quantized to fp8 e4m3
    (x 2^17) and streamed as 16 x 1MiB slabs issued up-front on two DMA
    queues; h3 quantized on-chip to fp8 (x 2^7); PE consumes weight
    chunk-pairs in DoubleRow perf mode (W stationary [128,2,128], f
    moving [128,2,1]) accumulating y1 in PSUM; partials AllReduce'd, the
    tiny Wd2/Wd3/Wd4 layers computed replicated on every core.
  - All fp16/fp32 constants are packed into two DRAM tensors so each rep
    issues only 2 const DMAs + 16 slab DMAs; GCN-critical mid DMAs (cc
    staging, gather readback) ride the DVE queue so they never wait
    behind weight slabs.
"""

import numpy as np
import ml_dtypes

N_CORES = 8
N = 1000          # real nodes
P = 1024          # padded nodes
S = P // N_CORES  # nodes per core = 128
F = 128           # input features
D = 512           # GCN hidden
H = 256           # dense hidden

NPAIR = 256                 # (j, i') chunk pairs per core
PAIR_B = 512                # fp8 bytes/partition per pair: 2 half x 2 t x 128 m
N_SLAB = 16
PAIRS_PER_SLAB = NPAIR // N_SLAB     # 16
SLAB_B = PAIRS_PER_SLAB * PAIR_B     # 8192 bytes/partition
WQ_SCALE = 2.0 ** 17        # Wd1 fp8 scale
F_SCALE = 2.0 ** 7          # f fp8 scale
OUT_DESCALE = 1.0 / (WQ_SCALE * F_SCALE)

# fp16 const tensor column offsets (fp16 elements)
C16_XK = 0                    # 8 x [128,128]
C16_ATS = C16_XK + 8 * 128    # 8 x [128,128]
C16_W1 = C16_ATS + 8 * 128    # [128,512]
C16_W2 = C16_W1 + D           # 4 x [128,512]
C16_W3 = C16_W2 + 4 * D       # 4 x [128,512]
C16_WD2 = C16_W3 + 4 * D      # 2 x [128,256]
C16_WD3 = C16_WD2 + 2 * H     # 2 x [128,256]
C16_WD4 = C16_WD3 + 2 * H     # 2 x [128,1]
C16_ID = C16_WD4 + 2          # [128,128]
C16_W = C16_ID + 128

# fp32 const tensor column offsets
C32_B1 = 0
C32_B2 = C32_B1 + D
C32_B3 = C32_B2 + D
C32_BD = C32_B3 + D           # bd1c, bd2c, bd3c: 3 x [128,2]
C32_BD4 = C32_BD + 6          # [128,1] broadcast
C32_W = C32_BD4 + 1

_NC = None


def _build_nc(reps=1, mode="full"):
    import concourse.bacc as bacc
    import concourse.mybir as mybir
    import concourse.tile as tile

    f32 = mybir.dt.float32
    f16 = mybir.dt.float16
    f8 = mybir.dt.float8e4
    RG = [list(range(N_CORES))]
    DR = mybir.MatmulPerfMode.DoubleRow
    Tanh = mybir.ActivationFunctionType.Tanh
    Bypass = mybir.AluOpType.bypass
    Add = mybir.AluOpType.add

    nc = bacc.Bacc("TRN2", target_bir_lowering=False, debug=False,
                   num_devices=N_CORES)

    c16 = nc.dram_tensor("c16", [128, C16_W], f16, kind="ExternalInput")
    c32 = nc.dram_tensor("c32", [128, C32_W], f32, kind="ExternalInput")
    wd1q = nc.dram_tensor("wd1q", [128, NPAIR * PAIR_B], f8, kind="ExternalInput")
    out = nc.dram_tensor("out", [1, 1], f32, kind="ExternalOutput")

    with tile.TileContext(nc) as tc:
        with (
            tc.tile_pool(name="const", bufs=1) as cp,
            tc.tile_pool(name="slab", bufs=1) as sp,
            tc.tile_pool(name="work", bufs=2) as wk,
            tc.tile_pool(name="gath", bufs=2) as gp,
            tc.tile_pool(name="psA", bufs=2, space="PSUM") as ppA,
            tc.tile_pool(name="psH", bufs=2, space="PSUM") as ppH,
            tc.tile_pool(name="psY", bufs=1, space="PSUM") as ppY,
            tc.tile_pool(name="psT", bufs=1, space="PSUM") as ppT,
            tc.tile_pool(name="psD", bufs=1, space="PSUM") as ppD,
            tc.tile_pool(name="dram", bufs=1, space="DRAM") as dp,
        ):
          for _rep in range(reps):
            # ---- const + slab DMAs (2 + 16 transfers) ----
            c16_t = cp.tile([128, C16_W], f16, tag="c16")
            nc.sync.dma_start(c16_t[:], c16[:])
            c32_t = cp.tile([128, C32_W], f32, tag="c32")
            nc.scalar.dma_start(c32_t[:], c32[:])
            slabs = []
            if mode != "gcn":
                for g in range(N_SLAB):
                    st = sp.tile([128, SLAB_B], f8, tag=f"slab{g}")
                    eng = nc.sync if g % 2 == 0 else nc.scalar
                    eng.dma_start(st[:], wd1q[:, g * SLAB_B:(g + 1) * SLAB_B])
                    slabs.append(st)

            xk = lambda q: c16_t[:, C16_XK + q * 128: C16_XK + (q + 1) * 128]
            ats = lambda q: c16_t[:, C16_ATS + q * 128: C16_ATS + (q + 1) * 128]

            # ================= GCN (fp16) =================
            if mode != "head":
                # ---- layer 1 (local 128 dst nodes) ----
                ps_a = ppA.tile([128, S], f32, tag="ps_a")
                for q in range(8):
                    nc.tensor.matmul(ps_a[:], xk(q), ats(q),
                                     start=(q == 0), stop=(q == 7))
                a1 = wk.tile([128, S], f16, tag="a1")
                nc.vector.tensor_copy(a1[:], ps_a[:])
                ps_h1 = ppH.tile([128, D], f32, tag="ps_h")
                nc.tensor.matmul(ps_h1[:], a1[:], c16_t[:, C16_W1:C16_W1 + D],
                                 start=True, stop=True)
                hb1 = wk.tile([128, D], f16, tag="hb")
                nc.vector.tensor_add(hb1[:], ps_h1[:], c32_t[:, C32_B1:C32_B1 + D])
                hs1 = wk.tile([128, D], f16, tag="hs")
                nc.scalar.activation(hs1[:], hb1[:], Tanh)

                def gather(hs, nm):
                    cci = dp.tile([128, D], f16, tag=f"cci{nm}")
                    nc.gpsimd.dma_start(cci[:], hs[:])
                    cco = dp.tile([P, D], f16, tag=f"cco{nm}", addr_space="Shared")
                    nc.gpsimd.collective_compute(
                        "AllGather", Bypass, replica_groups=RG,
                        ins=[cci.opt()], outs=[cco.opt()])
                    hin = gp.tile([128, 8 * D], f16, tag="hin")
                    nc.gpsimd.dma_start(
                        hin[:], cco.rearrange("(q p) d -> p q d", p=128))
                    return hin

                def gcn_layer(hin, w_off, b_off):
                    a2 = wk.tile([128, 4 * S], f16, tag="a2")
                    for m in range(4):
                        ps = ppA.tile([128, S], f32, tag="ps_a")
                        for q in range(8):
                            nc.tensor.matmul(
                                ps[:], hin[:, q * D + m * 128: q * D + (m + 1) * 128],
                                ats(q), start=(q == 0), stop=(q == 7))
                        nc.vector.tensor_copy(a2[:, m * S:(m + 1) * S], ps[:])
                    ps_h = ppH.tile([128, D], f32, tag="ps_h")
                    for m in range(4):
                        nc.tensor.matmul(
                            ps_h[:], a2[:, m * S:(m + 1) * S],
                            c16_t[:, w_off + m * D: w_off + (m + 1) * D],
                            start=(m == 0), stop=(m == 3))
                    hb = wk.tile([128, D], f16, tag="hb")
                    nc.vector.tensor_add(hb[:], ps_h[:], c32_t[:, b_off:b_off + D])
                    hs = wk.tile([128, D], f16, tag="hs")
                    nc.scalar.activation(hs[:], hb[:], Tanh)
                    return hs

                hin1 = gather(hs1, "1")
                hs2 = gcn_layer(hin1, C16_W2, C32_B2)
                hin2 = gather(hs2, "2")
                hs3 = gcn_layer(hin2, C16_W3, C32_B3)

            if mode == "gcn":
                out_sb = wk.tile([1, 1], f32, tag="out_sb")
                nc.vector.tensor_copy(out_sb[:], hs3[:1, :1])
                nc.gpsimd.dma_start(out[:], out_sb[:])
                continue

            # ---- transpose + fp8 quantize f ----
            h3T = []
            if mode == "head":
                for j in range(4):
                    t8 = wk.tile([128, 128], f8, tag=f"h3T{j}", bufs=1)
                    nc.vector.memset(t8[:], 0.001)
                    h3T.append(t8)
            else:
                idap = c16_t[:, C16_ID:C16_ID + 128]
                for j in range(4):
                    pst = ppT.tile([128, 128], f16, tag="pst")
                    nc.tensor.transpose(pst[:], hs3[:, j * 128:(j + 1) * 128], idap)
                    t8 = wk.tile([128, 128], f8, tag=f"h3T{j}", bufs=1)
                    nc.scalar.mul(t8[:], pst[:], float(F_SCALE))
                    h3T.append(t8)

            # ================= dense head: y1 = f @ Wd1 =================
            ps0 = ppY.tile([128, 1], f32, tag="ps0")
            ps1 = ppY.tile([128, 1], f32, tag="ps1")
            psh = [ps0, ps1]
            for g in range(N_SLAB):
                st = slabs[g]
                for pi in range(PAIRS_PER_SLAB):
                    pg = g * PAIRS_PER_SLAB + pi
                    j, i = pg // 64, pg % 64
                    rhs = h3T[j][:, 2 * i:2 * i + 2].rearrange(
                        "k (t n) -> k t n", t=2)
                    for hf in range(2):
                        lhsT = st[:, pi * PAIR_B + hf * 256:
                                  pi * PAIR_B + (hf + 1) * 256].rearrange(
                            "k (t m) -> k t m", t=2)
                        nc.tensor.matmul(psh[hf][:], lhsT, rhs,
                                         start=(pg == 0), stop=(pg == NPAIR - 1),
                                         perf_mode=DR, skip_group_check=True)

            # descale partials, AllReduce-add across cores
            y1p = wk.tile([128, 2], f32, tag="y1p")
            nc.vector.tensor_scalar_mul(y1p[:, 0:1], ps0[:], OUT_DESCALE)
            nc.vector.tensor_scalar_mul(y1p[:, 1:2], ps1[:], OUT_DESCALE)
            ccyi = dp.tile([128, 2], f32, tag="ccyi")
            nc.gpsimd.dma_start(ccyi[:], y1p[:])
            ccyo = dp.tile([128, 2], f32, tag="ccyo", addr_space="Shared")
            nc.gpsimd.collective_compute(
                "AllReduce", Add, replica_groups=RG,
                ins=[ccyi.opt()], outs=[ccyo.opt()])
            y1s = wk.tile([128, 2], f32, tag="y1s")
            nc.gpsimd.dma_start(y1s[:], ccyo[:])

            # leaky(y + b): channel m*128+p lives at [p, m]
            def leaky_cols(dst_ap, src_ap, bias_ap, tg, w=2):
                t0 = wk.tile([128, w], f32, tag=f"lk0{tg}")
                nc.vector.tensor_add(t0[:], src_ap, bias_ap)
                t1 = wk.tile([128, w], f32, tag=f"lk1{tg}")
                nc.vector.tensor_scalar_mul(t1[:], t0[:], 0.1)
                nc.vector.tensor_max(dst_ap, t0[:], t1[:])

            y1c = wk.tile([128, 2], f16, tag="y1c")
            leaky_cols(y1c[:], y1s[:], c32_t[:, C32_BD:C32_BD + 2], "1")

            def dense(y_in, w_off, b_off, tg):
                y_out = wk.tile([128, 2], f16, tag=f"y{tg}")
                for m in range(2):
                    ps = ppD.tile([128, 1], f32, tag="ps_d")
                    for k in range(2):
                        nc.tensor.matmul(
                            ps[:],
                            c16_t[:, w_off + k * H + m * 128:
                                  w_off + k * H + (m + 1) * 128],
                            y_in[:, k:k + 1], start=(k == 0), stop=(k == 1))
                    leaky_cols(y_out[:, m:m + 1], ps[:],
                               c32_t[:, b_off + m:b_off + m + 1], f"{tg}{m}", w=1)
                return y_out

            y2c = dense(y1c, C16_WD2, C32_BD + 2, "2")
            y3c = dense(y2c, C16_WD3, C32_BD + 4, "3")

            ps_o = ppD.tile([1, 1], f32, tag="ps_d")
            for k in range(2):
                nc.tensor.matmul(ps_o[:], c16_t[:, C16_WD4 + k:C16_WD4 + k + 1],
                                 y3c[:, k:k + 1], start=(k == 0), stop=(k == 1))
            out_sb = wk.tile([1, 1], f32, tag="out_sb")
            nc.vector.tensor_add(out_sb[:], ps_o[:], c32_t[:1, C32_BD4:C32_BD4 + 1])
            nc.gpsimd.dma_start(out[:], out_sb[:])

    nc.compile()
    return nc


def _get_nc():
    global _NC
    if _NC is None:
        _NC = _build_nc()
    return _NC


def make_in_maps(inputs):
    """Host-side sharding / preprocessing. Returns per-core input dicts."""
    x = np.asarray(inputs["x"], dtype=np.float32)
    ei = np.asarray(inputs["edge_index"])
    W1 = np.asarray(inputs["W1"], np.float32)
    W2 = np.asarray(inputs["W2"], np.float32)
    W3 = np.asarray(inputs["W3"], np.float32)
    b1 = np.asarray(inputs["b1"], np.float32)
    b2 = np.asarray(inputs["b2"], np.float32)
    b3 = np.asarray(inputs["b3"], np.float32)
    Wd1 = np.asarray(inputs["Wd1"], np.float32)
    Wd2 = np.asarray(inputs["Wd2"], np.float32)
    Wd3 = np.asarray(inputs["Wd3"], np.float32)
    Wd4 = np.asarray(inputs["Wd4"], np.float32)
    bd1 = np.asarray(inputs["bd1"], np.float32)
    bd2 = np.asarray(inputs["bd2"], np.float32)
    bd3 = np.asarray(inputs["bd3"], np.float32)
    bd4 = np.asarray(inputs["bd4"], np.float32)

    # normalized adjacency with self loops (GCNConv)
    src = ei[0].astype(np.int64)
    dst = ei[1].astype(np.int64)
    loop = np.arange(N, dtype=np.int64)
    s_all = np.concatenate([src, loop])
    d_all = np.concatenate([dst, loop])
    deg = np.bincount(d_all, minlength=N).astype(np.float32)
    dinv = np.where(deg > 0, 1.0 / np.sqrt(deg), 0.0).astype(np.float32)
    wnorm = dinv[s_all] * dinv[d_all]
    A = np.zeros((N, N), np.float32)
    np.add.at(A, (d_all, s_all), wnorm)
    AT = np.zeros((P, P), np.float16)
    AT[:N, :N] = A.T.astype(np.float16)

    xk = np.zeros((P, F), np.float16)
    xk[:N] = x.astype(np.float16)

    f16 = np.float16

    # shared fp16 consts (ats block filled per core below)
    c16_base = np.zeros((128, C16_W), f16)
    for q in range(8):
        c16_base[:, C16_XK + q * 128:C16_XK + (q + 1) * 128] = xk[q * 128:(q + 1) * 128]
    c16_base[:, C16_W1:C16_W1 + D] = W1.astype(f16)
    for m in range(4):
        c16_base[:, C16_W2 + m * D:C16_W2 + (m + 1) * D] = \
            W2[m * 128:(m + 1) * 128].astype(f16)
        c16_base[:, C16_W3 + m * D:C16_W3 + (m + 1) * D] = \
            W3[m * 128:(m + 1) * 128].astype(f16)
    for k in range(2):
        c16_base[:, C16_WD2 + k * H:C16_WD2 + (k + 1) * H] = \
            Wd2[k * 128:(k + 1) * 128].astype(f16)
        c16_base[:, C16_WD3 + k * H:C16_WD3 + (k + 1) * H] = \
            Wd3[k * 128:(k + 1) * 128].astype(f16)
    c16_base[:, C16_WD4:C16_WD4 + 2] = Wd4.reshape(2, 128).T.astype(f16)
    c16_base[:, C16_ID:C16_ID + 128] = np.eye(128, dtype=f16)

    c32 = np.zeros((128, C32_W), np.float32)
    bb = lambda b: np.broadcast_to(b[None, :], (128, b.shape[0]))
    c32[:, C32_B1:C32_B1 + D] = bb(b1)
    c32[:, C32_B2:C32_B2 + D] = bb(b2)
    c32[:, C32_B3:C32_B3 + D] = bb(b3)
    c32[:, C32_BD:C32_BD + 2] = bd1.reshape(2, 128).T
    c32[:, C32_BD + 2:C32_BD + 4] = bd2.reshape(2, 128).T
    c32[:, C32_BD + 4:C32_BD + 6] = bd3.reshape(2, 128).T
    c32[:, C32_BD4] = bd4[0]

    # Wd1 -> fp8 e4m3 per-core slabs.
    # Per-core rows: node i in [0,128), feat = 128*j + p; chunk pair over
    # node parity t: lhsT[(j,i',half)] = rows 512*(2i'+t)+128j+p,
    # cols half*128+m.  Flat col = ((j*64+i')*2+half)*256 + t*128 + m.
    rows_per_core = P * D // N_CORES  # 65536
    Wd1p = np.zeros((P * D, H), np.float32)
    Wd1p[:N * D] = Wd1
    Wq = np.clip(Wd1p * WQ_SCALE, -240.0, 240.0).astype(ml_dtypes.float8_e4m3)

    in_maps = []
    for r in range(N_CORES):
        sl = Wq[r * rows_per_core:(r + 1) * rows_per_core]
        w6 = sl.reshape(64, 2, 4, 128, 2, 128)      # i', t, j, p, half, m
        wd1q = np.ascontiguousarray(
            w6.transpose(3, 2, 0, 4, 1, 5).reshape(128, NPAIR * PAIR_B))
        c16_r = c16_base.copy()
        for q in range(8):
            c16_r[:, C16_ATS + q * 128:C16_ATS + (q + 1) * 128] = \
                AT[q * 128:(q + 1) * 128, r * S:(r + 1) * S]
        in_maps.append({"c16": c16_r, "c32": c32, "wd1q": wd1q})
    return in_maps


def kernel(**inputs):
    from concourse.bass_utils import run_bass_kernel_spmd
    in_maps = make_in_maps(inputs)
    nc = _get_nc()
    res = run_bass_kernel_spmd(nc, in_maps, core_ids=list(range(N_CORES)))
    return np.asarray(res.results[0]["out"], np.float32).reshape(1)
